# revision 1
# baseline (speedup 1.0000x reference)
"""C3D loss kernel for Trainium2 (8 NeuronCores, Bass/Tile).

Sharding: pure data parallel over B*2 = 8 shards (each image split into
top/bottom 176-row halves). Each core computes a partial sum of the loss
numerator; host combines and divides by the valid count.

Layout: partitions = 122 column blocks of 10 pixels (3+3 col halo -> 16
stored cols per block); free dims = (rows, 16). Every spatial shift (the
5x5 window and the normal central differences) is a free-dim offset, which
keeps all engine accesses at partition start 0 (a hardware requirement).

Window phase: channel-stacked tensors (3 channels x ~41 blocks on
partitions, 3 column groups built by SBUF->SBUF DMA) let the per-channel
subtract/square/product run as single wide instructions; channel sums run
on the tensor engine via fp16 embedding matmuls into PSUM, and the 25-
offset per-pixel accumulation runs on the tensor engine too (identity
matmul, PSUM accumulate). Squared differences are scaled by 0.1 inside the
Square activation so they fit fp16 (exp scale compensates exactly).

Out-of-image semantics (must match the reference's zero-pad + `vs` mask):
normals come from zero-padded xyz; the window-phase pred cloud is then
overwritten at out-of-image rows/cols with a poison value (per-core row
strips + column strips, small DMAs) so exp underflows to exactly 0
wherever the reference's `vs` is 0.
"""
import sys

sys.path.insert(0, "/opt/trn_rl_repo")

import numpy as np
from contextlib import ExitStack

import bass_rust
import concourse.bass as bass
import concourse.tile as tile
from concourse import bacc, mybir
from concourse.bass_utils import run_bass_kernel_spmd

F32 = mybir.dt.float32
F16 = mybir.dt.float16
AF = mybir.ActivationFunctionType
ALU = mybir.AluOpType

B, H, W = 4, 352, 1216
R = 2
ELL = 0.05
INV2ELL2 = float(np.float32(1.0 / (2.0 * ELL * ELL)))   # 200.0
EPS = 1e-8
N_CORES = 8

SH = H // 2          # shard rows per core = 176
NT = 2               # row tiles per core
TR = SH // NT        # output rows per tile = 88
HH = TR // 2         # PSUM chunk rows = 44
RB = TR + 6          # stored rows per tile = 94
CB = 10              # cols per block
NB = 122             # blocks
BW = CB + 6          # stored cols per block = 16
SW = CB * (NB - 1) + BW   # slab width = 1226 (slab col j <-> image col j-3)
PZ = 2000.0          # poison depth; (0.1*(PZ-80))**2 ~ 3.7e4 fits fp16
SQS = 0.0625         # pre-scale (2^-4, exact) so fp16 sq diffs stay finite
EXS = float(INV2ELL2 / (SQS * SQS))    # exp scale compensation = 20000
LN14 = float(np.log(0.25))
GRP = [(0, 41), (41, 82), (82, 122)]   # column groups

_prog_cache = {}


def _ap3(base_ap, dims, offset_elems):
    v = base_ap.copy()
    v.ap = bass_rust.VecI64Pair(dims)
    v.offset = v.offset + offset_elems
    return v


def _build_program():
    nc = bacc.Bacc("TRN2", target_bir_lowering=False, debug=False,
                   num_devices=N_CORES)

    for v in (EPS, LN14):
        t = nc.alloc_sbuf_tensor(f"const-f32-{v}", [128, 1], F32)
        nc.gpsimd.memset(t.ap(), v)
        nc.const_aps.aps[(F32, v)] = t.ap()
    nc.all_engine_barrier()

    dp_d = nc.dram_tensor("dp", [SH + 6, SW], F32, kind="ExternalInput").ap()
    dg_d = nc.dram_tensor("dg", [SH + 6, SW], F32, kind="ExternalInput").ap()
    xy1_d = nc.dram_tensor("xy1", [3, SH + 6, SW], F32, kind="ExternalInput").ap()
    mk_d = nc.dram_tensor("mk", [SH, SW], F32, kind="ExternalInput").ap()
    st_d = nc.dram_tensor("strip", [2, 3, NB, 3, BW], F32,
                          kind="ExternalInput").ap()
    pz_d = nc.dram_tensor("pzc", [3, RB, 2], F32, kind="ExternalInput").ap()
    id_d = nc.dram_tensor("idm", [NB, NB], F16, kind="ExternalInput").ap()
    e16_d = nc.dram_tensor("emb16", [3, 123, NB], F16, kind="ExternalInput").ap()
    out_d = nc.dram_tensor("out", [128, NT], F32, kind="ExternalOutput").ap()

    def slab_view(dram_ap, row0, nrows):
        return _ap3(dram_ap, [[CB, NB], [SW, nrows], [1, BW]], row0 * SW)

    with tile.TileContext(nc) as tc, ExitStack() as ctx:
        pool = ctx.enter_context(tc.tile_pool(name="p", bufs=1))
        psum = ctx.enter_context(tc.tile_pool(name="ps", bufs=1, space="PSUM"))
        idt = pool.tile([NB, NB], F16, name="idt")
        nc.sync.dma_start(out=idt[:], in_=id_d[:])
        e16 = pool.tile([123, 3 * NB], F16, name="e16")
        for g in range(3):
            nc.sync.dma_start(out=e16[:, g * NB:(g + 1) * NB], in_=e16_d[g])

        for t in range(NT):
            r0 = t * TR

            # ---------------- input loads ----------------
            dpt = pool.tile([NB, RB, BW], F32, name="dpt")
            nc.sync.dma_start(out=dpt[:], in_=slab_view(dp_d, r0, RB))
            dgt = pool.tile([NB, RB, BW], F32, name="dgt")
            nc.sync.dma_start(out=dgt[:], in_=slab_view(dg_d, r0, RB))
            xy1t = [pool.tile([NB, RB, BW], F32, name=f"xy1t{c}") for c in range(3)]
            for c in range(3):
                nc.sync.dma_start(out=xy1t[c][:], in_=slab_view(xy1_d[c], r0, RB))
            mkt = pool.tile([NB, TR, CB], F32, name="mkt")
            nc.sync.dma_start(
                out=mkt[:],
                in_=_ap3(mk_d, [[CB, NB], [SW, TR], [1, CB]], r0 * SW + 3))

            # ---------------- xyz (zero-padded; feeds normals + stacking) ---
            xp = [pool.tile([NB, RB, BW], F32, name=f"xp{c}") for c in range(3)]
            xg = [pool.tile([NB, RB, BW], F32, name=f"xg{c}") for c in range(3)]
            for c in range(3):
                nc.vector.tensor_mul(xp[c][:], xy1t[c][:], dpt[:])
                nc.vector.tensor_mul(xg[c][:], xy1t[c][:], dgt[:])

            # ------- stacked window tensors (SBUF->SBUF DMA; runs alongside
            # normals since both only read xp/xg) -------
            xpw, xgsc = [], []
            for g, (b0, b1) in enumerate(GRP):
                nb = b1 - b0
                pp = 3 * nb
                xpg = pool.tile([pp, 92, 14], F32, name=f"xpw{g}")
                xgg = pool.tile([pp, TR, CB], F32, name=f"xgs{g}")
                for c in range(3):
                    nc.sync.dma_start(out=xpg[c * nb:(c + 1) * nb],
                                      in_=xp[c][b0:b1, 1:93, 1:15])
                    nc.sync.dma_start(out=xgg[c * nb:(c + 1) * nb],
                                      in_=xg[c][b0:b1, 3:3 + TR, 3:3 + CB])
                # poison: per-core row strips at out-of-image rows
                if t == 0:
                    for c in range(3):
                        nc.sync.dma_start(out=xpg[c * nb:(c + 1) * nb, 0:2, :],
                                          in_=st_d[0, c, b0:b1, 1:3, 1:15])
                if t == NT - 1:
                    for c in range(3):
                        nc.sync.dma_start(
                            out=xpg[c * nb:(c + 1) * nb, 90:92, :],
                            in_=st_d[1, c, b0:b1, 0:2, 1:15])
                # poison: out-of-image columns (image cols -2,-1 / 1216,1217)
                if g == 0:
                    nc.sync.dma_start(
                        out=_ap3(xpg, [[nb * 92 * 14, 3], [14, 92], [1, 2]], 0),
                        in_=pz_d[:, 0:92, :])
                if g == 2:
                    nc.sync.dma_start(
                        out=_ap3(xpg, [[nb * 92 * 14, 3], [14, 92], [1, 2]],
                                 (nb - 1) * 92 * 14 + 8),
                        in_=pz_d[:, 0:92, :])
                nc.vector.tensor_scalar_mul(xpg[:], xpg[:], SQS)
                nc.vector.tensor_scalar_mul(xgg[:], xgg[:], SQS)
                xpw.append(xpg)
                xgsc.append(xgg)

            # ---------------- normals ----------------
            def w3(x, dr, dc):
                return x[:, 1 + dr:93 + dr, 1 + dc:15 + dc]

            nrm = {}
            for key, xc in (("p", xp), ("g", xg)):
                eng = nc.vector if key == "p" else nc.gpsimd
                gx = [pool.tile([NB, 92, 14], F32, name=f"gx{c}") for c in range(3)]
                gy = [pool.tile([NB, 92, 14], F32, name=f"gy{c}") for c in range(3)]
                for c in range(3):
                    nc.vector.tensor_sub(gx[c][:], w3(xc[c], 0, 1),
                                         w3(xc[c], 0, -1))
                    nc.vector.tensor_sub(gy[c][:], w3(xc[c], 1, 0),
                                         w3(xc[c], -1, 0))
                cr = [pool.tile([NB, 92, 14], F32, name=f"cr{c}") for c in range(3)]
                tA = pool.tile([NB, 92, 14], F32, name="tA")
                for c in range(3):
                    a, b = (c + 1) % 3, (c + 2) % 3
                    nc.vector.tensor_mul(cr[c][:], gx[a][:], gy[b][:])
                    eng.tensor_mul(tA[:], gx[b][:], gy[a][:])
                    eng.tensor_sub(cr[c][:], cr[c][:], tA[:])
                q = pool.tile([NB, 92, 14], F32, name="q")
                sqt = pool.tile([NB, 92, 14], F32, name="sqt", tag="tA")
                nc.scalar.activation(q[:], cr[0][:], AF.Square)
                nc.scalar.activation(sqt[:], cr[1][:], AF.Square)
                eng.tensor_add(q[:], q[:], sqt[:])
                nc.scalar.activation(sqt[:], cr[2][:], AF.Square)
                eng.tensor_add(q[:], q[:], sqt[:])
                # w = 0.25/(0.25*sqrt(q) + EPS), matching n/(|n|+eps)
                nc.scalar.activation(q[:], q[:], AF.Sqrt, scale=0.0625)
                nc.scalar.activation(q[:], q[:], AF.Ln, bias=EPS)
                nc.scalar.activation(q[:], q[:], AF.Exp, scale=-1.0, bias=LN14)
                nt_ = [pool.tile([NB, 92, 14], F16, name=f"n{key}{c}")
                       for c in range(3)]
                for c in range(3):
                    nc.vector.tensor_mul(nt_[c][:], cr[c][:], q[:])
                nrm[key] = nt_
            npn, ngn = nrm["p"], nrm["g"]

            # ------- stacked normals (after normals complete) -------
            nps, ngsc = [], []
            for g, (b0, b1) in enumerate(GRP):
                nb = b1 - b0
                pp = 3 * nb
                npg = pool.tile([pp, 92, 14], F16, name=f"nps{g}")
                ngg = pool.tile([pp, TR, CB], F16, name=f"ngs{g}")
                for c in range(3):
                    nc.sync.dma_start(out=npg[c * nb:(c + 1) * nb],
                                      in_=npn[c][b0:b1])
                    nc.sync.dma_start(out=ngg[c * nb:(c + 1) * nb],
                                      in_=ngn[c][b0:b1, 2:2 + TR, 2:2 + CB])
                nps.append(npg)
                ngsc.append(ngg)

            # ---------------- window phase ----------------
            accP = [psum.tile([NB, HH, CB], F32, name=f"accP{ch}") for ch in range(2)]
            ndP = psum.tile([NB, 2, 512], F32, name="ndP")

            def shs(x, dy, dx):
                return x[:, 2 + dy:2 + TR + dy, 2 + dx:2 + CB + dx]

            noff = (2 * R + 1) ** 2
            offs = [(dy, dx) for dy in range(-R, R + 1) for dx in range(-R, R + 1)]
            for oi, (dy, dx) in enumerate(offs):
                d2P = psum.tile([NB, 2, 512], F32, name="d2P", tag="d2P",
                                bufs=2)
                kgt = pool.tile([NB, TR, CB], F16, name="kgt", tag="kgt")
                stt = pool.tile([NB, TR, CB], F16, name="stt", tag="stt")
                trm = pool.tile([NB, TR, CB], F16, name="trm", tag="trm")
                sbs = [pool.tile([123, TR, CB], F16, name=f"sbf{g}",
                                 tag=f"sbf{g}", bufs=2) for g in range(3)]
                npr = [pool.tile([123, TR, CB], F16, name=f"npr{g}",
                                 tag=f"npr{g}", bufs=2) for g in range(3)]
                for g, (b0, b1) in enumerate(GRP):
                    pp = 3 * (b1 - b0)
                    seng = nc.gpsimd if g == 2 else nc.vector
                    seng.tensor_sub(sbs[g][0:pp], shs(xpw[g], dy, dx),
                                    xgsc[g][:])
                    if g == 2:
                        nc.vector.tensor_mul(sbs[g][0:pp], sbs[g][0:pp],
                                             sbs[g][0:pp])
                    else:
                        nc.scalar.activation(sbs[g][0:pp], sbs[g][0:pp],
                                             AF.Square)
                    nc.vector.tensor_mul(npr[g][0:pp], shs(nps[g], dy, dx),
                                         ngsc[g][:])
                for g in range(3):
                    pp = 3 * (GRP[g][1] - GRP[g][0])
                    for ch in range(2):
                        rs = slice(ch * HH, (ch + 1) * HH)
                        nc.tensor.matmul(d2P[:, ch, 0:HH * CB]
                                         .rearrange("p (r c) -> p r c", c=CB),
                                         e16[0:pp, g * NB:(g + 1) * NB],
                                         sbs[g][0:pp, rs, :],
                                         start=(g == 0), stop=(g == 2))
                    for ch in range(2):
                        rs = slice(ch * HH, (ch + 1) * HH)
                        nc.tensor.matmul(ndP[:, ch, 0:HH * CB]
                                         .rearrange("p (r c) -> p r c", c=CB),
                                         e16[0:pp, g * NB:(g + 1) * NB],
                                         npr[g][0:pp, rs, :],
                                         start=(g == 0), stop=(g == 2))
                nc.scalar.activation(
                    kgt[:].rearrange("p (a r) c -> p a (r c)", a=2),
                    d2P[:, :, 0:HH * CB], AF.Exp, scale=-EXS)
                nc.scalar.activation(
                    stt[:].rearrange("p (a r) c -> p a (r c)", a=2),
                    ndP[:, :, 0:HH * CB], AF.Abs)
                nc.gpsimd.tensor_scalar(stt[:], stt[:], 1.9, 0.1,
                                        ALU.mult, ALU.add)
                nc.vector.tensor_mul(trm[:], stt[:], kgt[:])
                for ch in range(2):
                    rs = slice(ch * HH, (ch + 1) * HH)
                    nc.tensor.matmul(accP[ch][:], idt[:], trm[:, rs, :],
                                     start=(oi == 0), stop=(oi == noff - 1))

            # ---------------- masked reduction ----------------
            nc.vector.tensor_mul(mkt[:, 0:HH, :], accP[0][:], mkt[:, 0:HH, :])
            nc.vector.tensor_mul(mkt[:, HH:TR, :], accP[1][:], mkt[:, HH:TR, :])
            red = pool.tile([NB, 1], F32, name="red")
            nc.vector.tensor_reduce(red[:], mkt[:], mybir.AxisListType.XY,
                                    ALU.add)
            nc.sync.dma_start(out=out_d[0:NB, t:t + 1], in_=red[:])

    nc.compile()
    return nc


def _consts():
    idm = np.eye(NB, dtype=np.float16)
    e = np.zeros((3, 123, NB), dtype=np.float16)
    for g, (b0, b1) in enumerate(GRP):
        nb = b1 - b0
        for c in range(3):
            for b in range(nb):
                e[g, c * nb + b, b0 + b] = 1.0
    return idm, e


def _strips(xy1_b, dp_b, r0_img):
    """Window-phase xp values for slab rows [0:3) and [179:182)."""
    out = np.zeros((2, 3, NB, 3, BW), dtype=np.float32)
    for side, base in ((0, r0_img - 3), (1, r0_img + SH)):
        vals = np.full((3, 3, SW), PZ, dtype=np.float32)
        for i in range(3):
            y = base + i
            if 0 <= y < H:
                row = np.full((3, SW), PZ, dtype=np.float32)
                row[:, 3:3 + W] = xy1_b[:, y, :] * dp_b[y, :]
                row[:, 1:3] = PZ
                row[:, 3 + W:3 + W + 2] = PZ
                vals[:, i, :] = row
        for p in range(NB):
            out[side, :, p, :, :] = vals[:, :, CB * p:CB * p + BW]
    return out


def kernel(depth_pred, depth_gt, xy1_grid, K, mask):
    if "nc" not in _prog_cache:
        _prog_cache["nc"] = _build_program()
    nc = _prog_cache["nc"]

    dp = np.asarray(depth_pred, dtype=np.float32).reshape(B, H, W)
    dg = np.asarray(depth_gt, dtype=np.float32).reshape(B, H, W)
    xy1 = np.asarray(xy1_grid, dtype=np.float32)
    mk = np.asarray(mask).reshape(B, H, W)

    idm, e16 = _consts()
    pzc = np.full((3, RB, 2), PZ, dtype=np.float32)
    in_maps = []
    for core in range(N_CORES):
        b, half = core // 2, core % 2
        r0 = half * SH
        lo, hi = r0 - 3, r0 + SH + 3
        slo, shi = max(lo, 0), min(hi, H)
        dps = np.zeros((SH + 6, SW), dtype=np.float32)
        dgs = np.zeros((SH + 6, SW), dtype=np.float32)
        xys = np.zeros((3, SH + 6, SW), dtype=np.float32)
        dps[slo - lo:shi - lo, 3:3 + W] = dp[b, slo:shi]
        dgs[slo - lo:shi - lo, 3:3 + W] = dg[b, slo:shi]
        xys[:, slo - lo:shi - lo, 3:3 + W] = xy1[b, :, slo:shi]
        mks = np.zeros((SH, SW), dtype=np.float32)
        mks[:, 3:3 + W] = mk[b, r0:r0 + SH]
        in_maps.append({
            "dp": dps, "dg": dgs, "xy1": xys, "mk": mks,
            "strip": _strips(xy1[b], dp[b], r0),
            "pzc": pzc, "idm": idm, "emb16": e16,
        })

    res = run_bass_kernel_spmd(nc, in_maps, list(range(N_CORES)))
    total = 0.0
    for core in range(N_CORES):
        total += res.results[core]["out"][0:NB, :].astype(np.float64).sum()
    nval = float(mk.sum(dtype=np.float64))
    return np.float32(-total / (nval + EPS))



# revision 5
# speedup vs baseline: 4.3407x; 4.3407x over previous
"""C3D loss kernel for Trainium2 (8 NeuronCores, Bass/Tile) — sparse design.

The mask is ~5% dense and every term of the loss is gated by mask[p], so the
kernel computes only at valid gt pixels. The host enumerates valid pixels
(np.nonzero — selection only, no arithmetic), splits the point list evenly
across the 8 cores, and ships per-point 7x7 patches of the raw inputs
(xy1, depth_pred) plus 3x3 depth_gt patches. Each core then computes, per
point: the pred 3D cloud on its 7x7 patch, pred normals at the 25 window
positions, the gt point + normal at the center, and the 5x5 correlation —
all as dense [128 x points] SIMD ops, ~25x less element volume than the
dense formulation.

Out-of-image semantics match the reference exactly: patches are zero-padded
(so normals see the reference's zero-pad), and an additive per-(point,offset)
term AD (+inf where p+delta falls outside the image, or for list-padding
dummy points) kills those window positions via exp -> 0. d2 is clamped below
f16-inf before the exp so the activation never sees inf.

Two custom DVE ops (registered into concourse.dve_ops at import):
  SQDIFF_ANT: out = (in0 - in1)^2      — fused window subtract+square
  TRMF_ANT:   out = (|in0|*s1 + imm2)*in1 — fused abs/coef/product
"""
import sys

sys.path.insert(0, "/opt/trn_rl_repo")

import numpy as np
from contextlib import ExitStack

import bass_rust
import concourse.bass as bass
import concourse.tile as tile
from concourse import bacc, mybir
from concourse.bass_utils import run_bass_kernel_spmd

# ---- custom DVE ops ------------------------------------------------------
from concourse.dve_spec import Spec, Src0, Src1, sq, maxx
from concourse.dve_ops import (
    DveOp, OPS, CUSTOM_DVE_SPECS, _SUB_OPCODE_FOR_NAME, _CUSTOM_DVE_ROW_BASE)

from concourse.dve_spec import C0, C1, C2

SQDIFF_ANT = DveOp(
    "SQDIFF_ANT",
    Spec(body=sq(Src0 - Src1),
         reference=lambda in0, in1, s0, s1, imm2:
         (in0.astype(np.float32) - in1.astype(np.float32)) ** 2),
    subdim=False,
    uops_sha={"v3": "eed49934a849c087", "v4": "cee42896e85173b8"},
)
TRMF_ANT = DveOp(
    "TRMF_ANT",
    Spec(body=(maxx(Src0, Src0 * C0) * C1 + C2) * Src1,
         reference=lambda in0, in1, s0, s1, imm2:
         (np.maximum(in0.astype(np.float32), in0 * s0) * s1 + imm2) * in1),
    subdim=False,
    uops_sha={"v3": "ea1d71b4cc9f3c9b", "v4": "12ffb22d2c2515d5"},
)

for _op in (SQDIFF_ANT, TRMF_ANT):
    if _op.name not in _SUB_OPCODE_FOR_NAME:
        OPS.append(_op)
        _SUB_OPCODE_FOR_NAME[_op.name] = _CUSTOM_DVE_ROW_BASE + len(OPS) - 1
    CUSTOM_DVE_SPECS[_op.name] = _op.spec

F32 = mybir.dt.float32
F16 = mybir.dt.float16
AF = mybir.ActivationFunctionType
ALU = mybir.AluOpType

B, H, W = 4, 352, 1216
INV2ELL2 = float(np.float32(1.0 / (2.0 * 0.05 * 0.05)))   # 200.0
EPS = 1e-8
N_CORES = 8

NV = 12288        # points per core (capacity; mean valid/core ~10.7k)
NCH = 3           # chunks
CK = NV // NCH // 128   # points per partition per chunk = 32
D2CLAMP = 59000.0

_prog_cache = {}


def _g7(ap_):
    """[128, CK, 49] channel plane -> [128, CK, 7, 7] grid view."""
    return ap_.rearrange("p n (a b) -> p n a b", a=7)


def _build_program():
    nc = bacc.Bacc("TRN2", target_bir_lowering=False, debug=False,
                   num_devices=N_CORES)
    for v in (0.0, EPS):
        t = nc.alloc_sbuf_tensor(f"const-f32-{v}", [128, 1], F32)
        nc.gpsimd.memset(t.ap(), v)
        nc.const_aps.aps[(F32, v)] = t.ap()
    nc.all_engine_barrier()

    p7_d = nc.dram_tensor("p7", [NCH, 128, CK, 4, 49], F32,
                          kind="ExternalInput").ap()
    d9_d = nc.dram_tensor("d9", [NCH, 128, CK, 9], F32,
                          kind="ExternalInput").ap()
    ad_d = nc.dram_tensor("ad", [NCH, 128, CK, 25], F16,
                          kind="ExternalInput").ap()
    out_d = nc.dram_tensor("out", [128, NCH], F32, kind="ExternalOutput").ap()

    with tile.TileContext(nc) as tc, ExitStack() as ctx:
        pool = ctx.enter_context(tc.tile_pool(name="p", bufs=1))

        for k in range(NCH):
            p7 = pool.tile([128, CK, 4, 49], F32, name="p7", tag="p7", bufs=2)
            nc.sync.dma_start(out=p7[:], in_=p7_d[k])
            d9 = pool.tile([128, CK, 9], F32, name="d9", tag="d9", bufs=2)
            nc.sync.dma_start(out=d9[:], in_=d9_d[k])
            ad = pool.tile([128, CK, 25], F16, name="ad", tag="ad", bufs=2)
            nc.sync.dma_start(out=ad[:], in_=ad_d[k])

            # ---- pred cloud on the 7x7 patch (f32, exact) ----
            xpp = pool.tile([128, CK, 3, 49], F32, name="xpp", tag="xpp")
            for c, eng in ((0, nc.vector), (1, nc.gpsimd), (2, nc.gpsimd)):
                eng.tensor_mul(xpp[:, :, c, :], p7[:, :, c, :], p7[:, :, 3, :])

            # ---- pred gradients at the 25 window positions (f16) ----
            gx = pool.tile([128, CK, 3, 25], F16, name="gx", tag="gx")
            gy = pool.tile([128, CK, 3, 25], F16, name="gy", tag="gy")
            for c in range(3):
                g7 = _g7(xpp[:, :, c, :])
                o5x = gx[:, :, c, :].rearrange("p n (a b) -> p n a b", a=5)
                o5y = gy[:, :, c, :].rearrange("p n (a b) -> p n a b", a=5)
                eng = nc.gpsimd if c == 2 else nc.vector
                eng.tensor_sub(o5x, g7[:, :, 1:6, 2:7], g7[:, :, 1:6, 0:5])
                nc.vector.tensor_sub(o5y, g7[:, :, 2:7, 1:6], g7[:, :, 0:5, 1:6])
            # keep cross products inside f16 range; normalization absorbs it
            nc.vector.tensor_scalar_mul(gy[:], gy[:], 0.25)

            # ---- cross products -> pred normal direction ----
            t1 = pool.tile([128, CK, 3, 25], F16, name="t1", tag="t1")
            crp = pool.tile([128, CK, 3, 25], F16, name="crp", tag="crp")
            for c in range(3):
                a, b = (c + 1) % 3, (c + 2) % 3
                eng = nc.gpsimd if c == 1 else nc.vector
                eng.tensor_mul(crp[:, :, c, :], gx[:, :, a, :], gy[:, :, b, :])
                eng2 = nc.gpsimd if c == 2 else nc.vector
                eng2.tensor_mul(t1[:, :, c, :], gx[:, :, b, :], gy[:, :, a, :])
            nc.vector.tensor_sub(crp[:], crp[:], t1[:])

            # q = |n|^2 (f32; crp*0.25 == reference n), rs = 1/(|n| + eps)
            sq3 = pool.tile([128, CK, 3, 25], F32, name="sq3", tag="sq3")
            nc.scalar.activation(sq3[:], crp[:], AF.Square)
            qq = pool.tile([128, CK, 25], F32, name="qq", tag="qq")
            nc.vector.tensor_add(qq[:], sq3[:, :, 0, :], sq3[:, :, 1, :])
            nc.vector.tensor_add(qq[:], qq[:], sq3[:, :, 2, :])
            nc.scalar.activation(qq[:], qq[:], AF.Sqrt)
            nc.scalar.activation(qq[:], qq[:], AF.Ln, bias=EPS)
            # rs stays f32: with q == 0 (all-zero pad windows) rs = 1/eps =
            # 1e8, which overflows f16 and would turn 0 * inf into NaN.
            rs = pool.tile([128, CK, 25], F32, name="rs", tag="rs")
            nc.scalar.activation(rs[:], qq[:], AF.Exp, scale=-1.0)
            npp = pool.tile([128, CK, 3, 25], F16, name="npp", tag="npp")
            nc.vector.tensor_mul(
                npp[:], crp[:],
                rs[:].unsqueeze(2).broadcast_to([128, CK, 3, 25]))

            # ---- gt cloud on the 3x3 patch, gt normal at center ----
            xgp = pool.tile([128, CK, 3, 9], F32, name="xgp", tag="xgp")
            for c in range(3):
                c33 = _g7(p7[:, :, c, :])[:, :, 2:5, 2:5]
                d33 = d9[:].rearrange("p n (a b) -> p n a b", a=3)
                nc.gpsimd.tensor_mul(
                    xgp[:, :, c, :].rearrange("p n (a b) -> p n a b", a=3),
                    c33, d33)
            ggx = pool.tile([128, CK, 3], F16, name="ggx", tag="ggx")
            ggy = pool.tile([128, CK, 3], F16, name="ggy", tag="ggy")
            nc.vector.tensor_sub(ggx[:], xgp[:, :, :, 5], xgp[:, :, :, 3])
            nc.vector.tensor_sub(ggy[:], xgp[:, :, :, 7], xgp[:, :, :, 1])
            nc.vector.tensor_scalar_mul(ggy[:], ggy[:], 0.25)
            crg = pool.tile([128, CK, 3], F16, name="crg", tag="crg")
            t2 = pool.tile([128, CK, 3], F16, name="t2", tag="t2")
            for c in range(3):
                a, b = (c + 1) % 3, (c + 2) % 3
                nc.vector.tensor_mul(crg[:, :, c], ggx[:, :, a], ggy[:, :, b])
                nc.vector.tensor_mul(t2[:, :, c], ggx[:, :, b], ggy[:, :, a])
            nc.vector.tensor_sub(crg[:], crg[:], t2[:])
            sqg = pool.tile([128, CK, 3], F32, name="sqg", tag="sqg")
            nc.scalar.activation(sqg[:], crg[:], AF.Square)
            qg = pool.tile([128, CK, 1], F32, name="qg", tag="qg")
            nc.vector.tensor_add(qg[:, :, 0], sqg[:, :, 0], sqg[:, :, 1])
            nc.vector.tensor_add(qg[:, :, 0], qg[:, :, 0], sqg[:, :, 2])
            nc.scalar.activation(qg[:], qg[:], AF.Sqrt)
            nc.scalar.activation(qg[:], qg[:], AF.Ln, bias=EPS)
            rsg = pool.tile([128, CK, 1], F32, name="rsg", tag="rsg")
            nc.scalar.activation(rsg[:], qg[:], AF.Exp, scale=-1.0)
            ngc = pool.tile([128, CK, 3], F16, name="ngc", tag="ngc")
            nc.vector.tensor_mul(ngc[:], crg[:],
                                 rsg[:].broadcast_to([128, CK, 3]))

            # ---- window: d2 and kernel ----
            sqd = pool.tile([128, CK, 3, 25], F16, name="sqd", tag="sqd")
            for c in range(3):
                g7 = _g7(xpp[:, :, c, :])
                xgc_b = xgp[:, :, c, 4:5].broadcast_to([128, CK, 5])
                for dy in range(5):
                    nc.vector._custom_dve(
                        SQDIFF_ANT,
                        out=sqd[:, :, c, 5 * dy:5 * dy + 5],
                        in0=g7[:, :, 1 + dy, 1:6],
                        in1=xgc_b)
            d2 = pool.tile([128, CK, 25], F16, name="d2", tag="d2")
            t3 = pool.tile([128, CK, 25], F16, name="t3", tag="t3")
            nc.vector.tensor_add(d2[:], sqd[:, :, 0, :], sqd[:, :, 1, :])
            nc.gpsimd.tensor_add(t3[:], sqd[:, :, 2, :], ad[:])
            nc.vector.tensor_add(d2[:], d2[:], t3[:])
            nc.vector.tensor_scalar(d2[:], d2[:], D2CLAMP, None, ALU.min)
            kg = pool.tile([128, CK, 25], F16, name="kg", tag="kg")
            nc.scalar.activation(kg[:], d2[:], AF.Exp, scale=-INV2ELL2)

            # ---- normal kernel: nd = <np, ng> ----
            ndp = pool.tile([128, CK, 25], F16, name="ndp", tag="ndp")
            t4 = pool.tile([128, CK, 25], F16, name="t4", tag="t4")
            nc.vector.tensor_mul(
                ndp[:], npp[:, :, 0, :],
                ngc[:, :, 0:1].broadcast_to([128, CK, 25]))
            nc.gpsimd.tensor_mul(
                t4[:], npp[:, :, 1, :],
                ngc[:, :, 1:2].broadcast_to([128, CK, 25]))
            nc.vector.tensor_add(ndp[:], ndp[:], t4[:])
            nc.vector.tensor_mul(
                t4[:], npp[:, :, 2, :],
                ngc[:, :, 2:3].broadcast_to([128, CK, 25]))
            nc.vector.tensor_add(ndp[:], ndp[:], t4[:])

            # ---- trm = (|nd|*1.9 + 0.1) * kg ; reduce ----
            trm = pool.tile([128, CK, 25], F16, name="trm", tag="trm")
            nc.vector._custom_dve(
                TRMF_ANT,
                out=trm[:].rearrange("p n x -> p (n x)"),
                in0=ndp[:].rearrange("p n x -> p (n x)"),
                in1=kg[:].rearrange("p n x -> p (n x)"),
                s0=-1.0, s1=1.9, imm2=0.1)
            red = pool.tile([128, 1], F32, name="red", tag="red", bufs=2)
            nc.vector.tensor_reduce(red[:], trm[:], mybir.AxisListType.XY,
                                    ALU.add)
            nc.sync.dma_start(out=out_d[:, k:k + 1], in_=red[:])

    nc.compile()
    return nc


def _host_prep(dp, dg, xy1, mk):
    """Compact valid pixels into per-core patch arrays."""
    NVT = N_CORES * NV
    # zero-padded fields (pad 3): channels = xy1 x/y/z, dp
    A4 = np.zeros((B, 4, H + 6, W + 6), dtype=np.float32)
    A4[:, 0:3, 3:3 + H, 3:3 + W] = xy1
    A4[:, 3, 3:3 + H, 3:3 + W] = dp
    DG = np.zeros((B, H + 2, W + 2), dtype=np.float32)
    DG[:, 1:1 + H, 1:1 + W] = dg

    bs, ys, xs = np.nonzero(mk)
    n = len(bs)
    assert n <= NVT, f"valid count {n} exceeds capacity {NVT}"
    pad = NVT - n
    bs = np.concatenate([bs, np.zeros(pad, dtype=bs.dtype)])
    ys = np.concatenate([ys, np.zeros(pad, dtype=ys.dtype)])
    xs = np.concatenate([xs, np.zeros(pad, dtype=xs.dtype)])

    r7 = np.arange(7)
    # P7[i] = A4[b, :, y-3+r : y+4, x-3+c : x+4]  (A4 is pre-shifted by 3)
    P7 = A4[bs[:, None, None], :, ys[:, None, None] + r7[None, :, None],
            xs[:, None, None] + r7[None, None, :]]          # [NVT,7,7,4]
    P7 = np.ascontiguousarray(P7.transpose(0, 3, 1, 2))      # [NVT,4,7,7]
    r3 = np.arange(3)
    D9 = DG[bs[:, None, None], ys[:, None, None] + r3[None, :, None],
            xs[:, None, None] + r3[None, None, :]]           # [NVT,3,3]

    dyx = np.arange(-2, 3)
    qy = ys[:, None, None] + dyx[None, :, None]              # [NVT,5,1]
    qx = xs[:, None, None] + dyx[None, None, :]
    ooi = (qy < 0) | (qy >= H) | (qx < 0) | (qx >= W)        # [NVT,5,5]
    ooi[n:] = True                                           # dummy points
    AD = np.where(ooi, np.inf, 0.0).astype(np.float16)

    P7 = P7.reshape(N_CORES, NCH, 128, CK, 4, 49)
    D9 = D9.reshape(N_CORES, NCH, 128, CK, 9)
    AD = AD.reshape(N_CORES, NCH, 128, CK, 25)
    return P7, D9, AD, n


def kernel(depth_pred, depth_gt, xy1_grid, K, mask):
    if "nc" not in _prog_cache:
        _prog_cache["nc"] = _build_program()
    nc = _prog_cache["nc"]

    dp = np.asarray(depth_pred, dtype=np.float32).reshape(B, H, W)
    dg = np.asarray(depth_gt, dtype=np.float32).reshape(B, H, W)
    xy1 = np.asarray(xy1_grid, dtype=np.float32)
    mk = np.asarray(mask).reshape(B, H, W)

    P7, D9, AD, n_valid = _host_prep(dp, dg, xy1, mk)
    in_maps = [{"p7": P7[c], "d9": D9[c], "ad": AD[c]}
               for c in range(N_CORES)]

    res = run_bass_kernel_spmd(nc, in_maps, list(range(N_CORES)))
    total = 0.0
    for core in range(N_CORES):
        total += res.results[core]["out"].astype(np.float64).sum()
    return np.float32(-total / (n_valid + EPS))
